# revision 16
# baseline (speedup 1.0000x reference)
"""MiniCPM (MLA-style) attention — Trainium2 Bass kernel, 8-way sharded.

Strategy (per spec sharding_hint, adapted for the MLA low-rank structure):
  - Phase A (sequence-parallel): each core computes the low-rank a-path for
    its 256-row block: q_a = hs @ wq_a -> rms_norm; ckv = hs @ wkv_a ->
    rms_norm(compressed) + RoPE(k_pe). Results are transposed on-chip (PE
    transpose) and AllGathered (bf16) so every core holds the full-length
    latent activations transposed: q_a_n^T [768,2048], ckv_n^T [256,2048],
    k_pe^T [32,2048].
  - Phase B/C (tensor-parallel over heads, 5 heads/core): Q^T/K^T/V built by
    bf16 matmuls directly in transposed layout; causal attention computed as
    S^T[k,q] tiles so softmax-normalized probs feed the PV matmul as the
    moving operand; a ones-column appended to V yields the softmax
    denominators for free in the same matmul. RoPE on q_pe is done in
    transposed layout using an extra set of column-swapped wq_b columns.
  - wo: resident in SBUF (bf16); each core computes a full [2048,2560]
    partial with its 320 rows of wo; host sums the 8 bf16 partials.

All matmuls run in bf16 (fp32 PSUM accumulate): full PE rate + FWL fast
weight loads, ~1-3e-3 end-to-end max relative error vs the fp32 reference.
Softmax skips max-subtraction: causal |scores| <= ~2.5.
"""

import sys
sys.path.insert(0, "/opt/trn_rl_repo")

from contextlib import ExitStack

import numpy as np

import concourse.bass as bass
import concourse.bacc as bacc
import concourse.tile as tile
from concourse import mybir
from concourse.bass_utils import run_bass_kernel_spmd
from concourse.masks import make_identity

F32 = mybir.dt.float32
BF16 = mybir.dt.bfloat16
AF = mybir.ActivationFunctionType

M = 8                  # cores
S = 2048               # sequence
H = 2560               # hidden
RB = S // M            # 256 rows per core (phase A)
QLR = 768              # q low rank
CKV = 256              # kv low rank (normed part)
QK_ROPE = 32
QK_NOPE = 64
Q_HEAD = 96
V_HEAD = 64
NH = 40
NHL = NH // M          # 5 heads per core
EPS = 1e-6
SM_SCALE = float(Q_HEAD) ** -0.5
NKT = S // 128         # 16 k-tiles
NQB = S // 512         # 4 q-blocks
VROW = NHL * (V_HEAD + 1)       # 325: per k-tile V' row layout (5x(64+ones))

_cache = {}


def _build():
    nc = bacc.Bacc(trn_type="TRN2", target_bir_lowering=False, debug=False,
                   num_devices=M)

    # ---- I/O ----
    hs_b = nc.dram_tensor("hs_b", [RB, H], BF16, kind="ExternalInput").ap()
    cosb = nc.dram_tensor("cosb", [RB, QK_ROPE], F32, kind="ExternalInput").ap()
    ssinb = nc.dram_tensor("ssinb", [RB, QK_ROPE], F32, kind="ExternalInput").ap()
    cosT = nc.dram_tensor("cosT", [QK_ROPE, S], F32, kind="ExternalInput").ap()
    ssinT = nc.dram_tensor("ssinT", [QK_ROPE, S], F32, kind="ExternalInput").ap()
    tri = nc.dram_tensor("tri", [128, 128], BF16, kind="ExternalInput").ap()
    # a/b weights arrive partition-major ([128, t, c]) so DMAs are contiguous
    wq_a = nc.dram_tensor("wq_a", [128, H // 128, QLR], BF16,
                          kind="ExternalInput").ap()
    wkv_a = nc.dram_tensor("wkv_a", [128, H // 128, CKV + QK_ROPE], BF16,
                           kind="ExternalInput").ap()
    wqb_l = nc.dram_tensor("wqb_l", [128, QLR // 128, NHL * 128], BF16,
                           kind="ExternalInput").ap()
    wkvk_l = nc.dram_tensor("wkvk_l", [128, CKV // 128, NHL * QK_NOPE], BF16,
                            kind="ExternalInput").ap()
    wkvv_l = nc.dram_tensor("wkvv_l", [128, CKV // 128, NHL * V_HEAD], BF16,
                            kind="ExternalInput").ap()
    wo_l = nc.dram_tensor("wo_l", [NHL * V_HEAD, H], BF16,
                          kind="ExternalInput").ap()
    out_p = nc.dram_tensor("out_p", [S, H], BF16, kind="ExternalOutput").ap()

    agin_kv = nc.dram_tensor("agin_kv", [CKV, RB], BF16,
                             kind="Internal").ap()
    agout_kv = nc.dram_tensor("agout_kv", [M * CKV, RB], BF16,
                              kind="Internal", addr_space="Shared").ap()
    agin_q = nc.dram_tensor("agin_q", [QLR + QK_ROPE, RB], BF16,
                            kind="Internal").ap()
    agout_q = nc.dram_tensor("agout_q", [M * (QLR + QK_ROPE), RB], BF16,
                             kind="Internal", addr_space="Shared").ap()
    agv_kv = agout_kv.rearrange("(r n) c -> n r c", r=M)
    agv_q = agout_q.rearrange("(r n) c -> n r c", r=M)

    with ExitStack() as ctx:
        tc = ctx.enter_context(tile.TileContext(nc))

        const = ctx.enter_context(tc.tile_pool(name="const", bufs=1))
        persist = ctx.enter_context(tc.tile_pool(name="persist", bufs=1))
        actx = ExitStack()
        sba = actx.enter_context(tc.tile_pool(name="sba", bufs=1))
        sbw = actx.enter_context(tc.tile_pool(name="sbw", bufs=2))
        ps = ctx.enter_context(tc.tile_pool(name="ps", bufs=2, space="PSUM"))

        # ---- phase-A constants (needed immediately) ----
        ident = const.tile([128, 128], BF16)
        make_identity(nc, ident)
        eps_t = const.tile([128, 1], F32)
        nc.vector.memset(eps_t, EPS)
        # natural-block cos/ssin [128, 2, 32]
        csb = const.tile([128, 2, 2 * QK_ROPE], F32)
        nc.sync.dma_start(out=csb[:, :, 0:QK_ROPE],
                          in_=cosb.rearrange("(t p) c -> p t c", p=128))
        nc.sync.dma_start(out=csb[:, :, QK_ROPE:],
                          in_=ssinb.rearrange("(t p) c -> p t c", p=128))

        # ---- persistent K^T and V' ----
        KT = [persist.tile([Q_HEAD, S], BF16, tag=f"KT{h}", name=f"KT{h}")
              for h in range(NHL)]
        Vp = persist.tile([128, NKT * VROW], BF16, tag="Vp")
        # ones columns of V' (once)
        nc.vector.memset(Vp, 1.0)

        # ================= PHASE A =================
        # hs^T first (gates everything)
        hsT = []
        for rt in range(2):
            hst = sba.tile([128, H], BF16, tag="hs", bufs=1, name=f"hs{rt}")
            nc.sync.dma_start(out=hst, in_=hs_b[128 * rt:128 * rt + 128, :])
            for hc in range(20):
                tp = ps.tile([128, 128], BF16, tag="work")
                nc.tensor.transpose(tp, hst[:, 128 * hc:128 * hc + 128], ident)
                t = sba.tile([128, 128], BF16, tag=f"hsT{rt}_{hc}",
                             name=f"hsT{rt}_{hc}")
                if hc % 2 == 0:
                    nc.vector.tensor_copy(t, tp)
                else:
                    nc.scalar.copy(t, tp)
                hsT.append(t)

        # ---- kv path first: ckv matmuls -> norm -> transpose -> AllGather
        # single-shot resident weight loads on separate queues (no chunk stalls)
        wkva_all = sba.tile([128, 20, CKV + QK_ROPE], BF16, tag="wkva_all",
                            bufs=1)
        nc.scalar.dma_start(out=wkva_all, in_=wkv_a)
        wqa_all = sba.tile([128, 20, QLR], BF16, tag="wqa_all", bufs=1)
        nc.gpsimd.dma_start(out=wqa_all, in_=wq_a)

        ckv_ps = [ps.tile([128, CKV + QK_ROPE], F32, tag=f"acc{4 + rt}", bufs=1,
                          name=f"ckv_ps{rt}")
                  for rt in range(2)]
        for hc in range(20):
            st, sp = hc == 0, hc == 19
            for rt in range(2):
                nc.tensor.matmul(ckv_ps[rt], hsT[rt * 20 + hc],
                                 wkva_all[:, hc, :], start=st, stop=sp)
        for rt in range(2):
            # --- ckv rms norm on first 256 cols ---
            sq3 = sbw.tile([128, CKV], F32, tag="sq", name=f"sq3_{rt}")
            ac = sbw.tile([128, 1], F32, tag="a0", name=f"ac_{rt}")
            nc.scalar.activation(sq3, ckv_ps[rt][:, 0:CKV], AF.Square, accum_out=ac)
            nc.scalar.activation(ac, ac, AF.Sqrt, bias=eps_t, scale=1.0 / CKV)
            crstd = sbw.tile([128, 1], F32, tag="a1", name=f"crstd_{rt}")
            nc.vector.reciprocal(crstd, ac)
            ckvn = sbw.tile([128, CKV], BF16, tag="ckvn", name=f"ckvn_{rt}")
            nc.vector.tensor_scalar_mul(ckvn, ckv_ps[rt][:, 0:CKV], crstd)
            for jc in range(2):
                tp = ps.tile([128, 128], BF16, tag="work", name=f"tpc_{rt}_{jc}")
                nc.tensor.transpose(tp, ckvn[:, 128 * jc:128 * jc + 128], ident)
                pc = sbw.tile([128, 128], BF16, tag="piece", name=f"pcc_{rt}_{jc}")
                nc.scalar.copy(pc, tp)
                nc.scalar.dma_start(out=agin_kv[128 * jc:128 * jc + 128,
                                              128 * rt:128 * rt + 128], in_=pc)

        nc.gpsimd.collective_compute(
            "AllGather", mybir.AluOpType.bypass,
            replica_groups=[list(range(M))],
            ins=[agin_kv], outs=[agout_kv],
        )

        # --- k_pe RoPE (natural) then transpose, into agin_q rows 768:800 ---
        for rt in range(2):
            t1 = sbw.tile([128, QK_ROPE], F32, tag="kp1", name=f"kp1_{rt}")
            nc.vector.tensor_mul(t1, ckv_ps[rt][:, CKV:CKV + QK_ROPE],
                                 csb[:, rt, 0:QK_ROPE])
            t2 = sbw.tile([128, QK_ROPE], F32, tag="kp2", name=f"kp2_{rt}")
            nc.vector.tensor_mul(t2[:, 0:16], ckv_ps[rt][:, CKV + 16:CKV + 32],
                                 csb[:, rt, QK_ROPE:QK_ROPE + 16])
            nc.vector.tensor_mul(t2[:, 16:32], ckv_ps[rt][:, CKV:CKV + 16],
                                 csb[:, rt, QK_ROPE + 16:QK_ROPE + 32])
            kpe = sbw.tile([128, QK_ROPE], BF16, tag="kp3", name=f"kp3_{rt}")
            nc.vector.tensor_add(kpe, t1, t2)
            tp = ps.tile([128, 128], BF16, tag="work", name=f"tpk_{rt}")
            nc.tensor.transpose(tp[0:QK_ROPE, :], kpe, ident)
            pc = sbw.tile([QK_ROPE, 128], BF16, tag="pieceb", name=f"pck_{rt}")
            nc.vector.tensor_copy(pc, tp[0:QK_ROPE, :])
            nc.scalar.dma_start(out=agin_q[QLR:QLR + QK_ROPE,
                                           128 * rt:128 * rt + 128], in_=pc)

        # ---- q path: qa matmuls -> norm -> transpose -> AllGather
        qa_ps = [[ps.tile([128, 384], F32, tag=f"acc{rt * 2 + jt}", bufs=1,
                          name=f"qa_ps{rt}{jt}")
                  for jt in range(2)] for rt in range(2)]
        for hc in range(20):
            st, sp = hc == 0, hc == 19
            for rt in range(2):
                for jt in range(2):
                    nc.tensor.matmul(
                        qa_ps[rt][jt], hsT[rt * 20 + hc],
                        wqa_all[:, hc, 384 * jt:384 * jt + 384],
                        start=st, stop=sp)

        for rt in range(2):
            # --- q_a rms norm (natural layout) ---
            sq = sbw.tile([128, 384], F32, tag="sq", name=f"sq_{rt}")
            a0 = sbw.tile([128, 1], F32, tag="a0", name=f"a0_{rt}")
            a1 = sbw.tile([128, 1], F32, tag="a1", name=f"a1_{rt}")
            nc.scalar.activation(sq, qa_ps[rt][0], AF.Square, accum_out=a0)
            sq2 = sbw.tile([128, 384], F32, tag="sq", name=f"sq2_{rt}")
            nc.scalar.activation(sq2, qa_ps[rt][1], AF.Square, accum_out=a1)
            ssum = sbw.tile([128, 1], F32, tag="a0", name=f"ssum_{rt}")
            nc.vector.tensor_add(ssum, a0, a1)
            nc.scalar.activation(ssum, ssum, AF.Sqrt, bias=eps_t, scale=1.0 / QLR)
            rstd = sbw.tile([128, 1], F32, tag="a1", name=f"rstd_{rt}")
            nc.vector.reciprocal(rstd, ssum)
            qan = sbw.tile([128, QLR], BF16, tag="qan", bufs=1, name=f"qan_{rt}")
            for jt in range(2):
                nc.vector.tensor_scalar_mul(qan[:, 384 * jt:384 * jt + 384],
                                            qa_ps[rt][jt], rstd)
            for jc in range(6):
                tp = ps.tile([128, 128], BF16, tag="work", name=f"tpq_{rt}_{jc}")
                nc.tensor.transpose(tp, qan[:, 128 * jc:128 * jc + 128], ident)
                pc = sbw.tile([128, 128], BF16, tag="pieceb",
                              name=f"pcq_{rt}_{jc}")
                if jc % 2 == 0:
                    nc.vector.tensor_copy(pc, tp)
                else:
                    nc.scalar.copy(pc, tp)
                nc.scalar.dma_start(out=agin_q[128 * jc:128 * jc + 128,
                                             128 * rt:128 * rt + 128], in_=pc)

        nc.gpsimd.collective_compute(
            "AllGather", mybir.AluOpType.bypass,
            replica_groups=[list(range(M))],
            ins=[agin_q], outs=[agout_q],
        )

        # ---- phase B/C weights: loaded while the AllGathers run ----
        tri_sb = const.tile([128, 128], BF16)
        nc.sync.dma_start(out=tri_sb, in_=tri)
        csT = const.tile([64, S], F32)
        nc.sync.dma_start(out=csT[0:32, :], in_=cosT)
        nc.sync.dma_start(out=csT[32:64, :], in_=ssinT)
        wkvk_sb = const.tile([128, 2, NHL * QK_NOPE], BF16)
        nc.sync.dma_start(out=wkvk_sb, in_=wkvk_l)
        wkvv_sb = const.tile([128, 2, NHL * V_HEAD], BF16)
        nc.sync.dma_start(out=wkvv_sb, in_=wkvv_l)
        wqb_sb = const.tile([128, 6, NHL * 128], BF16)
        nc.gpsimd.dma_start(out=wqb_sb, in_=wqb_l)
        wo01_sb = const.tile([128, 2, H], BF16)
        nc.gpsimd.dma_start(out=wo01_sb,
                            in_=wo_l[0:256].rearrange("(d p) c -> p d c", p=128))
        wo2_sb = const.tile([QK_NOPE, H], BF16)
        nc.gpsimd.dma_start(out=wo2_sb, in_=wo_l[256:320])

        actx.close()
        sbc = ctx.enter_context(tc.tile_pool(name="sbc", bufs=2))

        # ================= PHASE B: K^T and V' =================
        for kb in range(NQB):
            ckt = sbc.tile([128, 2, 2, RB], BF16, tag="latB", bufs=2,
                           name=f"ckt{kb}")
            for d in range(2):
                nc.sync.dma_start(
                    out=ckt[:, d],
                    in_=agv_kv[128 * d:128 * d + 128, 2 * kb:2 * kb + 2, :])
            cks = [ckt[:, 0], ckt[:, 1]]
            for h in range(NHL):
                kps = ps.tile([QK_NOPE, 512], F32, tag="work")
                for c in range(2):
                    nc.tensor.matmul(
                        kps,
                        wkvk_sb[:, c, QK_NOPE * h:QK_NOPE * h + QK_NOPE],
                        cks[c].rearrange("p r c -> p (r c)"),
                        start=(c == 0), stop=(c == 1))
                nc.scalar.copy(KT[h][0:QK_NOPE, 512 * kb:512 * kb + 512], kps)
                nc.gpsimd.dma_start(
                    out=KT[h][QK_NOPE:Q_HEAD, 512 * kb:512 * kb + 512]
                        .rearrange("p (r c) -> p r c", r=2),
                    in_=agv_q[QLR:QLR + QK_ROPE, 2 * kb:2 * kb + 2, :])
            for t4 in range(4):
                vps = ps.tile([128, NHL * V_HEAD], F32, tag="work")
                for c in range(2):
                    nc.tensor.matmul(
                        vps,
                        cks[c].rearrange("p r c -> p (r c)")
                              [:, 128 * t4:128 * t4 + 128],
                        wkvv_sb[:, c, :],
                        start=(c == 0), stop=(c == 1))
                kt = 4 * kb + t4
                vdst = bass.AP(tensor=Vp.tensor,
                               offset=Vp.offset + kt * VROW,
                               ap=[Vp.ap[0], [V_HEAD + 1, NHL], [1, V_HEAD]])
                nc.vector.tensor_copy(vdst, vps)

        # ================= PHASE C: per q-block =================
        QTs = {}
        LATs = {}

        def make_lat(qb):
            latt = sbc.tile([128, 6, 2, RB], BF16, tag="latC", bufs=2,
                            name=f"latt{qb}")
            for r in range(2):
                (nc.sync if r == 0 else nc.scalar).dma_start(
                    out=latt[:, :, r, :],
                    in_=agv_q[0:QLR].rearrange("(d p) r c -> p d r c", p=128)
                        [:, :, 2 * qb + r, :])
            LATs[qb] = [latt[:, c] for c in range(6)]
            QTs[qb] = []

        def make_qps(qb, h):
            lats = LATs[qb]
            wi = _wk[0]; _wk[0] += 1
            qps = ps.tile([128, 512], F32,
                          tag=("work" if wi % 3 < 2 else "acc5"),
                          bufs=(2 if wi % 3 < 2 else 1),
                          name=f"qps{qb}_{h}")
            for c in range(6):
                nc.tensor.matmul(
                    qps, wqb_sb[:, c, 128 * h:128 * h + 128],
                    lats[c].rearrange("p r c -> p (r c)"),
                    start=(c == 0), stop=(c == 5))
            return qps

        def extract_qt(qb, h, qps):
            qs = slice(512 * qb, 512 * qb + 512)
            qt = sbc.tile([Q_HEAD, 512], BF16, tag="QT", bufs=10,
                          name=f"qt{qb}_{h}")
            nc.scalar.copy(qt[0:QK_NOPE, :], qps[0:QK_NOPE, :])
            t1 = sbc.tile([QK_ROPE, 512], F32, tag="rp1", bufs=2,
                          name=f"rp1_{qb}_{h}")
            nc.vector.tensor_mul(t1, qps[64:96, :], csT[0:32, qs])
            t2 = sbc.tile([QK_ROPE, 512], F32, tag="rp2", bufs=2,
                          name=f"rp2_{qb}_{h}")
            nc.vector.tensor_mul(t2, qps[96:128, :], csT[32:64, qs])
            nc.vector.tensor_add(qt[QK_NOPE:Q_HEAD, :], t1, t2)
            QTs[qb].append(qt)

        def make_qt(qb):
            make_lat(qb)
            for h in range(NHL):
                extract_qt(qb, h, make_qps(qb, h))

        _wk = [0]
        make_qt(0)
        for qb in range(NQB):
            QT = QTs[qb]
            att = [ps.tile([VROW // NHL, 512], F32, tag=f"acc{h}", bufs=1,
                           name=f"att{h}")
                   for h in range(NHL)]
            nkt = 4 * qb + 4
            for kt in range(nkt):
                o = max(0, 128 * kt - 512 * qb)
                for h in range(NHL):
                    wi = _wk[0]; _wk[0] += 1
                    sps = ps.tile([128, 512], F32,
                                  tag=("work" if wi % 3 < 2 else "acc5"),
                                  bufs=(2 if wi % 3 < 2 else 1),
                                  name=f"sps{qb}_{kt}_{h}")
                    nc.tensor.matmul(sps[:, o:512],
                                     KT[h][:, 128 * kt:128 * kt + 128],
                                     QT[h][:, o:512],
                                     start=True, stop=True)
                    pt = sbc.tile([128, 512], BF16, tag="PT", bufs=4,
                                  name=f"pt{qb}_{kt}_{h}")
                    nc.scalar.activation(pt[:, o:512], sps[:, o:512],
                                         AF.Exp, scale=SM_SCALE)
                    if 128 * kt >= 512 * qb:
                        nc.vector.tensor_mul(pt[:, o:o + 128],
                                             pt[:, o:o + 128], tri_sb)
                    nc.tensor.matmul(att[h][:, o:512],
                                     Vp[:, kt * VROW + 65 * h:kt * VROW + 65 * h + 65],
                                     pt[:, o:512],
                                     start=(kt == 0), stop=(kt == nkt - 1),
                                     skip_group_check=True)

            if qb + 1 < NQB:
                make_lat(qb + 1)

            # epilogue: softmax denominators -> normalized aT (bf16).
            # Denominator copies ride the (idle) Scalar engine; DVE keeps the
            # reciprocal + normalize muls; next q-block's QT matmuls fill the
            # PE, with the DVE-side qt extraction deferred two heads so aT[0]
            # completes early and the wo matmuls can start.
            aT = [sbc.tile([128, 512], BF16, tag=f"aT{p}", bufs=2, name=f"aT{p}")
                  for p in range(2)]
            aT4 = sbc.tile([QK_NOPE, 512], BF16, tag="aT4", bufs=2)
            qps_next = []
            for h in range(NHL):
                rcs = sbc.tile([1, 512], F32, tag="rcs", bufs=2,
                               name=f"rcs{qb}_{h}")
                nc.scalar.copy(rcs, att[h][64:65, :])
                rc = sbc.tile([1, 512], F32, tag="rc", bufs=2,
                              name=f"rc{qb}_{h}")
                nc.vector.reciprocal_approx_fast(rc, rcs)
                bcst = sbc.tile([QK_NOPE, 512], F32, tag="bc", bufs=2,
                                name=f"bc{qb}_{h}")
                nc.gpsimd.partition_broadcast(bcst, rc)
                if h < 4:
                    dst = aT[h // 2][64 * (h % 2):64 * (h % 2) + 64, :]
                else:
                    dst = aT4
                nc.vector.tensor_mul(dst, att[h][0:64, :], bcst)
                if qb + 1 < NQB:
                    qps_next.append(make_qps(qb + 1, h))
                    if h >= 2:
                        extract_qt(qb + 1, h - 2, qps_next[h - 2])
            if qb + 1 < NQB:
                for h in range(NHL - 2, NHL):
                    extract_qt(qb + 1, h, qps_next[h])
            for hc in range(5):
                hcs = slice(512 * hc, 512 * hc + 512)
                w0, w1 = wo01_sb[:, 0, hcs], wo01_sb[:, 1, hcs]
                w2 = wo2_sb[:, hcs]
                for half in range(2):
                    osb = sbc.tile([128, 2, 512], BF16, tag="osb", bufs=2,
                                   name=f"osb{qb}_{hc}_{half}")
                    for qq in range(2):
                        qt4 = 2 * half + qq
                        qsl = slice(128 * qt4, 128 * qt4 + 128)
                        wi = _wk[0]; _wk[0] += 1
                        ops = ps.tile([128, 512], F32,
                                      tag=("work" if wi % 3 < 2 else "acc5"),
                                      bufs=(2 if wi % 3 < 2 else 1),
                                      name=f"ops{qb}_{hc}_{qt4}")
                        nc.tensor.matmul(ops, aT[0][:, qsl], w0,
                                         start=True, stop=False)
                        nc.tensor.matmul(ops, aT[1][:, qsl], w1,
                                         start=False, stop=False)
                        nc.tensor.matmul(ops, aT4[:, qsl], w2,
                                         start=False, stop=True)
                        if qt4 % 2 == 0:
                            nc.vector.tensor_copy(osb[:, qq, :], ops)
                        else:
                            nc.scalar.copy(osb[:, qq, :], ops)
                    (nc.sync if hc % 2 == 0 else nc.gpsimd).dma_start(
                        out=out_p.rearrange("(d p) c -> p d c", p=128)
                                 [:, 4 * qb + 2 * half:4 * qb + 2 * half + 2, hcs],
                        in_=osb)

    nc.compile()
    return nc


def _prep(inputs):
    import ml_dtypes
    BF = ml_dtypes.bfloat16
    hs = np.ascontiguousarray(np.asarray(inputs["hidden_states"], np.float32)[0])
    cos = np.asarray(inputs["cos"], np.float32)
    sin = np.asarray(inputs["sin"], np.float32)
    wq_a = np.asarray(inputs["wq_a"], np.float32)
    q_ln = np.asarray(inputs["q_a_ln_w"], np.float32)
    wq_b = np.asarray(inputs["wq_b"], np.float32)
    wkv_a = np.asarray(inputs["wkv_a"], np.float32)
    kv_ln = np.asarray(inputs["kv_a_ln_w"], np.float32)
    wkv_b = np.asarray(inputs["wkv_b"], np.float32)
    wo = np.asarray(inputs["wo"], np.float32)

    if not np.all(q_ln == 1.0):
        wq_b = wq_b * q_ln[:, None]
    if not np.all(kv_ln == 1.0):
        wkv_b = wkv_b * kv_ln[:, None]

    ssin = np.concatenate([-sin[:, :16], sin[:, 16:]], axis=1)
    cosT = np.ascontiguousarray(cos.T)
    ssinT = np.ascontiguousarray(ssin.T)
    tri = np.triu(np.ones((128, 128), np.float32)).astype(BF)
    hs_bf = hs.astype(BF)

    def pm(w, t):
        # [t*128, c] row-major -> partition-major [128, t, c]
        c = w.shape[1]
        return np.ascontiguousarray(
            w.reshape(t, 128, c).transpose(1, 0, 2))

    wq_a_bf = pm(wq_a.astype(BF), 20)
    wkv_a_bf = pm(wkv_a.astype(BF), 20)

    in_maps = []
    for c in range(M):
        heads = range(NHL * c, NHL * c + NHL)
        qb_cols = []
        for h in heads:
            qb_cols.extend(range(96 * h, 96 * h + 96))
            # swapped pe columns: [16:32] then [0:16] of the pe block
            qb_cols.extend(range(96 * h + 80, 96 * h + 96))
            qb_cols.extend(range(96 * h + 64, 96 * h + 80))
        wqb_loc = pm(np.ascontiguousarray(wq_b[:, qb_cols]).astype(BF), 6)
        kcols, vcols = [], []
        for h in heads:
            kcols.extend(range(128 * h, 128 * h + 64))
            vcols.extend(range(128 * h + 64, 128 * h + 128))
        in_maps.append({
            "hs_b": np.ascontiguousarray(hs_bf[RB * c:RB * c + RB]),
            "cosb": np.ascontiguousarray(cos[RB * c:RB * c + RB]),
            "ssinb": np.ascontiguousarray(ssin[RB * c:RB * c + RB]),
            "cosT": cosT,
            "ssinT": ssinT,
            "tri": tri,
            "wq_a": wq_a_bf,
            "wkv_a": wkv_a_bf,
            "wqb_l": wqb_loc,
            "wkvk_l": pm(np.ascontiguousarray(wkv_b[:, kcols]).astype(BF), 2),
            "wkvv_l": pm(np.ascontiguousarray(wkv_b[:, vcols]).astype(BF), 2),
            "wo_l": np.ascontiguousarray(
                wo[NHL * V_HEAD * c:NHL * V_HEAD * (c + 1)]).astype(BF),
        })
    return in_maps


def kernel(**inputs):
    if "nc" not in _cache:
        _cache["nc"] = _build()
    nc = _cache["nc"]
    in_maps = _prep(inputs)
    res = run_bass_kernel_spmd(nc, in_maps, core_ids=list(range(M)))
    out = res.results[0]["out_p"].astype(np.float32)
    for c in range(1, M):
        out += res.results[c]["out_p"].astype(np.float32)
    return out.reshape(1, S, H)


# revision 21
# speedup vs baseline: 1.0227x; 1.0227x over previous
"""MiniCPM (MLA-style) attention — Trainium2 Bass kernel, 8-way sharded.

Strategy (per spec sharding_hint, adapted for the MLA low-rank structure):
  - Phase A (sequence-parallel): each core computes the low-rank a-path for
    its 256-row block: q_a = hs @ wq_a -> rms_norm; ckv = hs @ wkv_a ->
    rms_norm(compressed) + RoPE(k_pe). Results are transposed on-chip (PE
    transpose) and AllGathered (bf16) so every core holds the full-length
    latent activations transposed: q_a_n^T [768,2048], ckv_n^T [256,2048],
    k_pe^T [32,2048].
  - Phase B/C (tensor-parallel over heads, 5 heads/core): Q^T/K^T/V built by
    bf16 matmuls directly in transposed layout; causal attention computed as
    S^T[k,q] tiles so softmax-normalized probs feed the PV matmul as the
    moving operand; a ones-column appended to V yields the softmax
    denominators for free in the same matmul. RoPE on q_pe is done in
    transposed layout using an extra set of column-swapped wq_b columns.
  - wo: resident in SBUF (bf16); each core computes a full [2048,2560]
    partial with its 320 rows of wo; host sums the 8 bf16 partials.

All matmuls run in bf16 (fp32 PSUM accumulate): full PE rate + FWL fast
weight loads, ~1-3e-3 end-to-end max relative error vs the fp32 reference.
Softmax skips max-subtraction: causal |scores| <= ~2.5.
"""

import sys
sys.path.insert(0, "/opt/trn_rl_repo")

from contextlib import ExitStack

import numpy as np

import concourse.bass as bass
import concourse.bacc as bacc
import concourse.tile as tile
from concourse import mybir
from concourse import bass_utils as _bu
from concourse.bass_utils import run_bass_kernel_spmd
from concourse.masks import make_identity



F32 = mybir.dt.float32
BF16 = mybir.dt.bfloat16
AF = mybir.ActivationFunctionType

M = 8                  # cores
S = 2048               # sequence
H = 2560               # hidden
RB = S // M            # 256 rows per core (phase A)
QLR = 768              # q low rank
CKV = 256              # kv low rank (normed part)
QK_ROPE = 32
QK_NOPE = 64
Q_HEAD = 96
V_HEAD = 64
NH = 40
NHL = NH // M          # 5 heads per core
EPS = 1e-6
SM_SCALE = float(Q_HEAD) ** -0.5
NKT = S // 128         # 16 k-tiles
NQB = S // 512         # 4 q-blocks
VROW = NHL * (V_HEAD + 1)       # 325: per k-tile V' row layout (5x(64+ones))

_cache = {}


def _build():
    nc = bacc.Bacc(trn_type="TRN2", target_bir_lowering=False, debug=False,
                   num_devices=M)

    # ---- I/O ----
    hs_b = nc.dram_tensor("hs_b", [RB, H], BF16, kind="ExternalInput").ap()
    csb_h = nc.dram_tensor("csb_h", [128, 2, 2 * QK_ROPE], F32,
                           kind="ExternalInput").ap()
    cosT = nc.dram_tensor("cosT", [QK_ROPE, S], F32, kind="ExternalInput").ap()
    ssinT = nc.dram_tensor("ssinT", [QK_ROPE, S], F32, kind="ExternalInput").ap()
    tri = nc.dram_tensor("tri", [128, 128], BF16, kind="ExternalInput").ap()
    # a/b weights arrive partition-major ([128, t, c]) so DMAs are contiguous
    wq_a = nc.dram_tensor("wq_a", [128, H // 128, QLR], BF16,
                          kind="ExternalInput").ap()
    wkv_a = nc.dram_tensor("wkv_a", [128, H // 128, CKV + QK_ROPE], BF16,
                           kind="ExternalInput").ap()
    wqb_l = nc.dram_tensor("wqb_l", [128, QLR // 128, NHL * 128], BF16,
                           kind="ExternalInput").ap()
    wkvk_l = nc.dram_tensor("wkvk_l", [128, CKV // 128, NHL * QK_NOPE], BF16,
                            kind="ExternalInput").ap()
    wkvv_l = nc.dram_tensor("wkvv_l", [128, CKV // 128, NHL * V_HEAD], BF16,
                            kind="ExternalInput").ap()
    wo_l = nc.dram_tensor("wo_l", [NHL * V_HEAD, H], BF16,
                          kind="ExternalInput").ap()
    out_p = nc.dram_tensor("out_p", [S, H], BF16, kind="ExternalOutput").ap()

    agin_kv = nc.dram_tensor("agin_kv", [CKV, RB], BF16,
                             kind="Internal").ap()
    agout_kv = nc.dram_tensor("agout_kv", [M * CKV, RB], BF16,
                              kind="Internal", addr_space="Shared").ap()
    agin_q = nc.dram_tensor("agin_q", [QLR + QK_ROPE, RB], BF16,
                            kind="Internal").ap()
    agout_q = nc.dram_tensor("agout_q", [M * (QLR + QK_ROPE), RB], BF16,
                             kind="Internal", addr_space="Shared").ap()
    agv_kv = agout_kv.rearrange("(r n) c -> n r c", r=M)
    agv_q = agout_q.rearrange("(r n) c -> n r c", r=M)

    with ExitStack() as ctx:
        tc = ctx.enter_context(tile.TileContext(nc))

        const = ctx.enter_context(tc.tile_pool(name="const", bufs=1))
        persist = ctx.enter_context(tc.tile_pool(name="persist", bufs=1))
        actx = ExitStack()
        sba = actx.enter_context(tc.tile_pool(name="sba", bufs=1))
        sbw = actx.enter_context(tc.tile_pool(name="sbw", bufs=2))
        ps = ctx.enter_context(tc.tile_pool(name="ps", bufs=2, space="PSUM"))

        # ---- phase-A constants (needed immediately) ----
        ident = const.tile([128, 128], BF16)
        make_identity(nc, ident)
        eps_t = const.tile([128, 1], F32)
        nc.vector.memset(eps_t, EPS)
        # natural-block cos/ssin [128, 2, 2*32], host-packed contiguous
        csb = const.tile([128, 2, 2 * QK_ROPE], F32)
        nc.scalar.dma_start(out=csb, in_=csb_h)

        # ---- persistent K^T and V' ----
        KT = [persist.tile([Q_HEAD, S], BF16, tag=f"KT{h}", name=f"KT{h}")
              for h in range(NHL)]
        Vp = persist.tile([128, NKT * VROW], BF16, tag="Vp")
        # ones columns of V' (once)
        nc.vector.memset(Vp, 1.0)

        # ================= PHASE A =================
        # hs^T first (gates everything)
        hsT = []
        for rt in range(2):
            hst = sba.tile([128, H], BF16, tag="hs", bufs=1, name=f"hs{rt}")
            nc.sync.dma_start(out=hst, in_=hs_b[128 * rt:128 * rt + 128, :])
            for hc in range(20):
                tp = ps.tile([128, 128], BF16, tag="work")
                nc.tensor.transpose(tp, hst[:, 128 * hc:128 * hc + 128], ident)
                t = sba.tile([128, 128], BF16, tag=f"hsT{rt}_{hc}",
                             name=f"hsT{rt}_{hc}")
                if hc % 2 == 0:
                    nc.vector.tensor_copy(t, tp)
                else:
                    nc.scalar.copy(t, tp)
                hsT.append(t)

        # ---- kv path first: ckv matmuls -> norm -> transpose -> AllGather
        # single-shot resident weight loads on separate queues (no chunk stalls)
        wkva_all = sba.tile([128, 20, CKV + QK_ROPE], BF16, tag="wkva_all",
                            bufs=1)
        nc.scalar.dma_start(out=wkva_all, in_=wkv_a)
        wqa_all = sba.tile([128, 20, QLR], BF16, tag="wqa_all", bufs=1)
        nc.gpsimd.dma_start(out=wqa_all, in_=wq_a)

        ckv_ps = [ps.tile([128, CKV + QK_ROPE], F32, tag=f"acc{4 + rt}", bufs=1,
                          name=f"ckv_ps{rt}")
                  for rt in range(2)]
        for hc in range(20):
            st, sp = hc == 0, hc == 19
            for rt in range(2):
                nc.tensor.matmul(ckv_ps[rt], hsT[rt * 20 + hc],
                                 wkva_all[:, hc, :], start=st, stop=sp)
        for rt in range(2):
            # --- ckv rms norm on first 256 cols ---
            sq3 = sbw.tile([128, CKV], F32, tag="sq", name=f"sq3_{rt}")
            ac = sbw.tile([128, 1], F32, tag="a0", name=f"ac_{rt}")
            nc.scalar.activation(sq3, ckv_ps[rt][:, 0:CKV], AF.Square, accum_out=ac)
            nc.scalar.activation(ac, ac, AF.Sqrt, bias=eps_t, scale=1.0 / CKV)
            crstd = sbw.tile([128, 1], F32, tag="a1", name=f"crstd_{rt}")
            nc.vector.reciprocal(crstd, ac)
            ckvn = sbw.tile([128, CKV], BF16, tag="ckvn", name=f"ckvn_{rt}")
            nc.vector.tensor_scalar_mul(ckvn, ckv_ps[rt][:, 0:CKV], crstd)
            for jc in range(2):
                tp = ps.tile([128, 128], BF16, tag="work", name=f"tpc_{rt}_{jc}")
                nc.tensor.transpose(tp, ckvn[:, 128 * jc:128 * jc + 128], ident)
                pc = sbw.tile([128, 128], BF16, tag="piece", name=f"pcc_{rt}_{jc}")
                nc.scalar.copy(pc, tp)
                nc.scalar.dma_start(out=agin_kv[128 * jc:128 * jc + 128,
                                              128 * rt:128 * rt + 128], in_=pc)

        nc.gpsimd.collective_compute(
            "AllGather", mybir.AluOpType.bypass,
            replica_groups=[list(range(M))],
            ins=[agin_kv], outs=[agout_kv],
        )

        # --- k_pe RoPE (natural) then transpose, into agin_q rows 768:800 ---
        for rt in range(2):
            t1 = sbw.tile([128, QK_ROPE], F32, tag="kp1", name=f"kp1_{rt}")
            nc.vector.tensor_mul(t1, ckv_ps[rt][:, CKV:CKV + QK_ROPE],
                                 csb[:, rt, 0:QK_ROPE])
            t2 = sbw.tile([128, QK_ROPE], F32, tag="kp2", name=f"kp2_{rt}")
            nc.vector.tensor_mul(t2[:, 0:16], ckv_ps[rt][:, CKV + 16:CKV + 32],
                                 csb[:, rt, QK_ROPE:QK_ROPE + 16])
            nc.vector.tensor_mul(t2[:, 16:32], ckv_ps[rt][:, CKV:CKV + 16],
                                 csb[:, rt, QK_ROPE + 16:QK_ROPE + 32])
            kpe = sbw.tile([128, QK_ROPE], BF16, tag="kp3", name=f"kp3_{rt}")
            nc.vector.tensor_add(kpe, t1, t2)
            tp = ps.tile([128, 128], BF16, tag="work", name=f"tpk_{rt}")
            nc.tensor.transpose(tp[0:QK_ROPE, :], kpe, ident)
            pc = sbw.tile([QK_ROPE, 128], BF16, tag="pieceb", name=f"pck_{rt}")
            nc.vector.tensor_copy(pc, tp[0:QK_ROPE, :])
            nc.scalar.dma_start(out=agin_q[QLR:QLR + QK_ROPE,
                                           128 * rt:128 * rt + 128], in_=pc)

        # ---- q path: qa matmuls -> norm -> transpose -> AllGather
        qa_ps = [[ps.tile([128, 384], F32, tag=f"acc{rt * 2 + jt}", bufs=1,
                          name=f"qa_ps{rt}{jt}")
                  for jt in range(2)] for rt in range(2)]
        for hc in range(20):
            st, sp = hc == 0, hc == 19
            for rt in range(2):
                for jt in range(2):
                    nc.tensor.matmul(
                        qa_ps[rt][jt], hsT[rt * 20 + hc],
                        wqa_all[:, hc, 384 * jt:384 * jt + 384],
                        start=st, stop=sp)

        for rt in range(2):
            # --- q_a rms norm (natural layout) ---
            sq = sbw.tile([128, 384], F32, tag="sq", name=f"sq_{rt}")
            a0 = sbw.tile([128, 1], F32, tag="a0", name=f"a0_{rt}")
            a1 = sbw.tile([128, 1], F32, tag="a1", name=f"a1_{rt}")
            nc.scalar.activation(sq, qa_ps[rt][0], AF.Square, accum_out=a0)
            sq2 = sbw.tile([128, 384], F32, tag="sq", name=f"sq2_{rt}")
            nc.scalar.activation(sq2, qa_ps[rt][1], AF.Square, accum_out=a1)
            ssum = sbw.tile([128, 1], F32, tag="a0", name=f"ssum_{rt}")
            nc.vector.tensor_add(ssum, a0, a1)
            nc.scalar.activation(ssum, ssum, AF.Sqrt, bias=eps_t, scale=1.0 / QLR)
            rstd = sbw.tile([128, 1], F32, tag="a1", name=f"rstd_{rt}")
            nc.vector.reciprocal(rstd, ssum)
            qan = sbw.tile([128, QLR], BF16, tag="qan", bufs=1, name=f"qan_{rt}")
            for jt in range(2):
                nc.vector.tensor_scalar_mul(qan[:, 384 * jt:384 * jt + 384],
                                            qa_ps[rt][jt], rstd)
            for jc in range(6):
                tp = ps.tile([128, 128], BF16, tag="work", name=f"tpq_{rt}_{jc}")
                nc.tensor.transpose(tp, qan[:, 128 * jc:128 * jc + 128], ident)
                pc = sbw.tile([128, 128], BF16, tag="pieceb",
                              name=f"pcq_{rt}_{jc}")
                if jc % 2 == 0:
                    nc.vector.tensor_copy(pc, tp)
                else:
                    nc.scalar.copy(pc, tp)
                nc.scalar.dma_start(out=agin_q[128 * jc:128 * jc + 128,
                                             128 * rt:128 * rt + 128], in_=pc)

        nc.gpsimd.collective_compute(
            "AllGather", mybir.AluOpType.bypass,
            replica_groups=[list(range(M))],
            ins=[agin_q], outs=[agout_q],
        )

        # ---- phase B/C weights: loaded while the AllGathers run ----
        tri_sb = const.tile([128, 128], BF16)
        nc.sync.dma_start(out=tri_sb, in_=tri)
        csT = const.tile([64, S], F32)
        nc.sync.dma_start(out=csT[0:32, :], in_=cosT)
        nc.sync.dma_start(out=csT[32:64, :], in_=ssinT)
        wkvk_sb = const.tile([128, 2, NHL * QK_NOPE], BF16)
        nc.sync.dma_start(out=wkvk_sb, in_=wkvk_l)
        wkvv_sb = const.tile([128, 2, NHL * V_HEAD], BF16)
        nc.sync.dma_start(out=wkvv_sb, in_=wkvv_l)
        wqb_sb = const.tile([128, 6, NHL * 128], BF16)
        nc.gpsimd.dma_start(out=wqb_sb, in_=wqb_l)
        wo01_sb = const.tile([128, 2, H], BF16)
        nc.gpsimd.dma_start(out=wo01_sb,
                            in_=wo_l[0:256].rearrange("(d p) c -> p d c", p=128))
        wo2_sb = const.tile([QK_NOPE, H], BF16)
        nc.gpsimd.dma_start(out=wo2_sb, in_=wo_l[256:320])

        actx.close()
        sbc = ctx.enter_context(tc.tile_pool(name="sbc", bufs=2))

        # ================= PHASE B: K^T and V' =================
        for kb in range(NQB):
            ckt = sbc.tile([128, 2, 2, RB], BF16, tag="latB", bufs=2,
                           name=f"ckt{kb}")
            for d in range(2):
                nc.sync.dma_start(
                    out=ckt[:, d],
                    in_=agv_kv[128 * d:128 * d + 128, 2 * kb:2 * kb + 2, :])
            cks = [ckt[:, 0], ckt[:, 1]]
            for h in range(NHL):
                kps = ps.tile([QK_NOPE, 512], F32, tag="work")
                for c in range(2):
                    nc.tensor.matmul(
                        kps,
                        wkvk_sb[:, c, QK_NOPE * h:QK_NOPE * h + QK_NOPE],
                        cks[c].rearrange("p r c -> p (r c)"),
                        start=(c == 0), stop=(c == 1))
                nc.scalar.copy(KT[h][0:QK_NOPE, 512 * kb:512 * kb + 512], kps)
                nc.gpsimd.dma_start(
                    out=KT[h][QK_NOPE:Q_HEAD, 512 * kb:512 * kb + 512]
                        .rearrange("p (r c) -> p r c", r=2),
                    in_=agv_q[QLR:QLR + QK_ROPE, 2 * kb:2 * kb + 2, :])
            for t4 in range(4):
                vps = ps.tile([128, NHL * V_HEAD], F32, tag="work")
                for c in range(2):
                    nc.tensor.matmul(
                        vps,
                        cks[c].rearrange("p r c -> p (r c)")
                              [:, 128 * t4:128 * t4 + 128],
                        wkvv_sb[:, c, :],
                        start=(c == 0), stop=(c == 1))
                kt = 4 * kb + t4
                vdst = bass.AP(tensor=Vp.tensor,
                               offset=Vp.offset + kt * VROW,
                               ap=[Vp.ap[0], [V_HEAD + 1, NHL], [1, V_HEAD]])
                nc.vector.tensor_copy(vdst, vps)

        # ================= PHASE C: per q-block =================
        QTs = {}
        LATs = {}

        def make_lat(qb):
            latt = sbc.tile([128, 6, 2, RB], BF16, tag="latC", bufs=2,
                            name=f"latt{qb}")
            for r in range(2):
                (nc.sync if r == 0 else nc.scalar).dma_start(
                    out=latt[:, :, r, :],
                    in_=agv_q[0:QLR].rearrange("(d p) r c -> p d r c", p=128)
                        [:, :, 2 * qb + r, :])
            LATs[qb] = [latt[:, c] for c in range(6)]
            QTs[qb] = []

        def make_qps(qb, h):
            lats = LATs[qb]
            wi = _wk[0]; _wk[0] += 1
            qps = ps.tile([128, 512], F32,
                          tag=("work" if wi % 3 < 2 else "acc5"),
                          bufs=(2 if wi % 3 < 2 else 1),
                          name=f"qps{qb}_{h}")
            for c in range(6):
                nc.tensor.matmul(
                    qps, wqb_sb[:, c, 128 * h:128 * h + 128],
                    lats[c].rearrange("p r c -> p (r c)"),
                    start=(c == 0), stop=(c == 5))
            return qps

        def extract_qt(qb, h, qps):
            qs = slice(512 * qb, 512 * qb + 512)
            qt = sbc.tile([Q_HEAD, 512], BF16, tag="QT", bufs=10,
                          name=f"qt{qb}_{h}")
            nc.scalar.copy(qt[0:QK_NOPE, :], qps[0:QK_NOPE, :])
            t1 = sbc.tile([QK_ROPE, 512], F32, tag="rp1", bufs=2,
                          name=f"rp1_{qb}_{h}")
            nc.vector.tensor_mul(t1, qps[64:96, :], csT[0:32, qs])
            t2 = sbc.tile([QK_ROPE, 512], F32, tag="rp2", bufs=2,
                          name=f"rp2_{qb}_{h}")
            nc.vector.tensor_mul(t2, qps[96:128, :], csT[32:64, qs])
            nc.vector.tensor_add(qt[QK_NOPE:Q_HEAD, :], t1, t2)
            QTs[qb].append(qt)

        def make_qt(qb):
            make_lat(qb)
            for h in range(NHL):
                extract_qt(qb, h, make_qps(qb, h))

        _wk = [0]
        make_qt(0)
        for qb in range(NQB):
            QT = QTs[qb]
            att = [ps.tile([VROW // NHL, 512], F32, tag=f"acc{h}", bufs=1,
                           name=f"att{h}")
                   for h in range(NHL)]
            nkt = 4 * qb + 4
            for kt in range(nkt):
                o = max(0, 128 * kt - 512 * qb)
                for h in range(NHL):
                    wi = _wk[0]; _wk[0] += 1
                    sps = ps.tile([128, 512], F32,
                                  tag=("work" if wi % 3 < 2 else "acc5"),
                                  bufs=(2 if wi % 3 < 2 else 1),
                                  name=f"sps{qb}_{kt}_{h}")
                    nc.tensor.matmul(sps[:, o:512],
                                     KT[h][:, 128 * kt:128 * kt + 128],
                                     QT[h][:, o:512],
                                     start=True, stop=True)
                    pt = sbc.tile([128, 512], BF16, tag="PT", bufs=4,
                                  name=f"pt{qb}_{kt}_{h}")
                    nc.scalar.activation(pt[:, o:512], sps[:, o:512],
                                         AF.Exp, scale=SM_SCALE)
                    if 128 * kt >= 512 * qb:
                        nc.vector.tensor_mul(pt[:, o:o + 128],
                                             pt[:, o:o + 128], tri_sb)
                    nc.tensor.matmul(att[h][:, o:512],
                                     Vp[:, kt * VROW + 65 * h:kt * VROW + 65 * h + 65],
                                     pt[:, o:512],
                                     start=(kt == 0), stop=(kt == nkt - 1),
                                     skip_group_check=True)

            if qb + 1 < NQB:
                make_lat(qb + 1)

            # epilogue: softmax denominators -> normalized aT (bf16).
            # Denominator copies ride the (idle) Scalar engine; DVE keeps the
            # reciprocal + normalize muls; next q-block's QT matmuls fill the
            # PE, with the DVE-side qt extraction deferred two heads so aT[0]
            # completes early and the wo matmuls can start.
            aT = [sbc.tile([128, 512], BF16, tag=f"aT{p}", bufs=2, name=f"aT{p}")
                  for p in range(2)]
            aT4 = sbc.tile([QK_NOPE, 512], BF16, tag="aT4", bufs=2)
            qps_next = []
            for h in range(NHL):
                rcs = sbc.tile([1, 512], F32, tag="rcs", bufs=2,
                               name=f"rcs{qb}_{h}")
                nc.scalar.copy(rcs, att[h][64:65, :])
                rc = sbc.tile([1, 512], F32, tag="rc", bufs=2,
                              name=f"rc{qb}_{h}")
                nc.vector.reciprocal_approx_fast(rc, rcs)
                bcst = sbc.tile([QK_NOPE, 512], F32, tag="bc", bufs=2,
                                name=f"bc{qb}_{h}")
                nc.gpsimd.partition_broadcast(bcst, rc)
                if h < 4:
                    dst = aT[h // 2][64 * (h % 2):64 * (h % 2) + 64, :]
                else:
                    dst = aT4
                nc.vector.tensor_mul(dst, att[h][0:64, :], bcst)
                if qb + 1 < NQB:
                    qps_next.append(make_qps(qb + 1, h))
                    if h >= 2:
                        extract_qt(qb + 1, h - 2, qps_next[h - 2])
            if qb + 1 < NQB:
                for h in range(NHL - 2, NHL):
                    extract_qt(qb + 1, h, qps_next[h])
            for hc in range(5):
                hcs = slice(512 * hc, 512 * hc + 512)
                w0, w1 = wo01_sb[:, 0, hcs], wo01_sb[:, 1, hcs]
                w2 = wo2_sb[:, hcs]
                for half in range(2):
                    osb = sbc.tile([128, 2, 512], BF16, tag="osb", bufs=2,
                                   name=f"osb{qb}_{hc}_{half}")
                    for qq in range(2):
                        qt4 = 2 * half + qq
                        qsl = slice(128 * qt4, 128 * qt4 + 128)
                        wi = _wk[0]; _wk[0] += 1
                        ops = ps.tile([128, 512], F32,
                                      tag=("work" if wi % 3 < 2 else "acc5"),
                                      bufs=(2 if wi % 3 < 2 else 1),
                                      name=f"ops{qb}_{hc}_{qt4}")
                        nc.tensor.matmul(ops, aT[0][:, qsl], w0,
                                         start=True, stop=False)
                        nc.tensor.matmul(ops, aT[1][:, qsl], w1,
                                         start=False, stop=False)
                        nc.tensor.matmul(ops, aT4[:, qsl], w2,
                                         start=False, stop=True)
                        if qt4 % 2 == 0:
                            nc.vector.tensor_copy(osb[:, qq, :], ops)
                        else:
                            nc.scalar.copy(osb[:, qq, :], ops)
                    (nc.sync if hc % 2 == 0 else nc.gpsimd).dma_start(
                        out=out_p.rearrange("(d p) c -> p d c", p=128)
                                 [:, 4 * qb + 2 * half:4 * qb + 2 * half + 2, hcs],
                        in_=osb)

    nc.compile()
    return nc


def _prep(inputs):
    import ml_dtypes
    BF = ml_dtypes.bfloat16
    hs = np.ascontiguousarray(np.asarray(inputs["hidden_states"], np.float32)[0])
    cos = np.asarray(inputs["cos"], np.float32)
    sin = np.asarray(inputs["sin"], np.float32)
    wq_a = np.asarray(inputs["wq_a"], np.float32)
    q_ln = np.asarray(inputs["q_a_ln_w"], np.float32)
    wq_b = np.asarray(inputs["wq_b"], np.float32)
    wkv_a = np.asarray(inputs["wkv_a"], np.float32)
    kv_ln = np.asarray(inputs["kv_a_ln_w"], np.float32)
    wkv_b = np.asarray(inputs["wkv_b"], np.float32)
    wo = np.asarray(inputs["wo"], np.float32)

    if not np.all(q_ln == 1.0):
        wq_b = wq_b * q_ln[:, None]
    if not np.all(kv_ln == 1.0):
        wkv_b = wkv_b * kv_ln[:, None]

    ssin = np.concatenate([-sin[:, :16], sin[:, 16:]], axis=1)
    cosT = np.ascontiguousarray(cos.T)
    ssinT = np.ascontiguousarray(ssin.T)
    tri = np.triu(np.ones((128, 128), np.float32)).astype(BF)
    hs_bf = hs.astype(BF)

    def pm(w, t):
        # [t*128, c] row-major -> partition-major [128, t, c]
        c = w.shape[1]
        return np.ascontiguousarray(
            w.reshape(t, 128, c).transpose(1, 0, 2))

    wq_a_bf = pm(wq_a.astype(BF), 20)
    wkv_a_bf = pm(wkv_a.astype(BF), 20)

    in_maps = []
    for c in range(M):
        heads = range(NHL * c, NHL * c + NHL)
        qb_cols = []
        for h in heads:
            qb_cols.extend(range(96 * h, 96 * h + 96))
            # swapped pe columns: [16:32] then [0:16] of the pe block
            qb_cols.extend(range(96 * h + 80, 96 * h + 96))
            qb_cols.extend(range(96 * h + 64, 96 * h + 80))
        wqb_loc = pm(np.ascontiguousarray(wq_b[:, qb_cols]).astype(BF), 6)
        kcols, vcols = [], []
        for h in heads:
            kcols.extend(range(128 * h, 128 * h + 64))
            vcols.extend(range(128 * h + 64, 128 * h + 128))
        csb_loc = np.empty((128, 2, 2 * QK_ROPE), np.float32)
        cb = cos[RB * c:RB * c + RB].reshape(2, 128, QK_ROPE)
        sb = ssin[RB * c:RB * c + RB].reshape(2, 128, QK_ROPE)
        csb_loc[:, :, :QK_ROPE] = cb.transpose(1, 0, 2)
        csb_loc[:, :, QK_ROPE:] = sb.transpose(1, 0, 2)
        in_maps.append({
            "hs_b": np.ascontiguousarray(hs_bf[RB * c:RB * c + RB]),
            "csb_h": csb_loc,
            "cosT": cosT,
            "ssinT": ssinT,
            "tri": tri,
            "wq_a": wq_a_bf,
            "wkv_a": wkv_a_bf,
            "wqb_l": wqb_loc,
            "wkvk_l": pm(np.ascontiguousarray(wkv_b[:, kcols]).astype(BF), 2),
            "wkvv_l": pm(np.ascontiguousarray(wkv_b[:, vcols]).astype(BF), 2),
            "wo_l": np.ascontiguousarray(
                wo[NHL * V_HEAD * c:NHL * V_HEAD * (c + 1)]).astype(BF),
        })
    return in_maps


def kernel(**inputs):
    if "nc" not in _cache:
        _cache["nc"] = _build()
    nc = _cache["nc"]
    in_maps = _prep(inputs)
    res = run_bass_kernel_spmd(nc, in_maps, core_ids=list(range(M)))
    out = res.results[0]["out_p"].astype(np.float32)
    for c in range(1, M):
        out += res.results[c]["out_p"].astype(np.float32)
    return out.reshape(1, S, H)


# revision 22
# speedup vs baseline: 1.0290x; 1.0062x over previous
"""MiniCPM (MLA-style) attention — Trainium2 Bass kernel, 8-way sharded.

Strategy (per spec sharding_hint, adapted for the MLA low-rank structure):
  - Phase A (sequence-parallel): each core computes the low-rank a-path for
    its 256-row block: q_a = hs @ wq_a -> rms_norm; ckv = hs @ wkv_a ->
    rms_norm(compressed) + RoPE(k_pe). Results are transposed on-chip (PE
    transpose) and AllGathered (bf16) so every core holds the full-length
    latent activations transposed: q_a_n^T [768,2048], ckv_n^T [256,2048],
    k_pe^T [32,2048].
  - Phase B/C (tensor-parallel over heads, 5 heads/core): Q^T/K^T/V built by
    bf16 matmuls directly in transposed layout; causal attention computed as
    S^T[k,q] tiles so softmax-normalized probs feed the PV matmul as the
    moving operand; a ones-column appended to V yields the softmax
    denominators for free in the same matmul. RoPE on q_pe is done in
    transposed layout using an extra set of column-swapped wq_b columns.
  - wo: resident in SBUF (bf16); each core computes a full [2048,2560]
    partial with its 320 rows of wo; host sums the 8 bf16 partials.

All matmuls run in bf16 (fp32 PSUM accumulate): full PE rate + FWL fast
weight loads, ~1-3e-3 end-to-end max relative error vs the fp32 reference.
Softmax skips max-subtraction: causal |scores| <= ~2.5.
"""

import sys
sys.path.insert(0, "/opt/trn_rl_repo")

from contextlib import ExitStack

import numpy as np

import concourse.bass as bass
import concourse.bacc as bacc
import concourse.tile as tile
from concourse import mybir
from concourse import bass_utils as _bu
from concourse.bass_utils import run_bass_kernel_spmd
from concourse.masks import make_identity



F32 = mybir.dt.float32
BF16 = mybir.dt.bfloat16
AF = mybir.ActivationFunctionType

M = 8                  # cores
S = 2048               # sequence
H = 2560               # hidden
RB = S // M            # 256 rows per core (phase A)
QLR = 768              # q low rank
CKV = 256              # kv low rank (normed part)
QK_ROPE = 32
QK_NOPE = 64
Q_HEAD = 96
V_HEAD = 64
NH = 40
NHL = NH // M          # 5 heads per core
EPS = 1e-6
SM_SCALE = float(Q_HEAD) ** -0.5
NKT = S // 128         # 16 k-tiles
NQB = S // 512         # 4 q-blocks
VROW = NHL * (V_HEAD + 1)       # 325: per k-tile V' row layout (5x(64+ones))

_cache = {}


def _build():
    nc = bacc.Bacc(trn_type="TRN2", target_bir_lowering=False, debug=False,
                   num_devices=M)

    # ---- I/O ----
    hs_b = nc.dram_tensor("hs_b", [RB, H], BF16, kind="ExternalInput").ap()
    csb_h = nc.dram_tensor("csb_h", [128, 2, 2 * QK_ROPE], F32,
                           kind="ExternalInput").ap()
    cosT = nc.dram_tensor("cosT", [QK_ROPE, S], F32, kind="ExternalInput").ap()
    ssinT = nc.dram_tensor("ssinT", [QK_ROPE, S], F32, kind="ExternalInput").ap()
    tri = nc.dram_tensor("tri", [128, 128], BF16, kind="ExternalInput").ap()
    # a/b weights arrive partition-major ([128, t, c]) so DMAs are contiguous
    wq_a = nc.dram_tensor("wq_a", [128, H // 128, QLR], BF16,
                          kind="ExternalInput").ap()
    wkv_a = nc.dram_tensor("wkv_a", [128, H // 128, CKV + QK_ROPE], BF16,
                           kind="ExternalInput").ap()
    wqb_l = nc.dram_tensor("wqb_l", [128, QLR // 128, NHL * 128], BF16,
                           kind="ExternalInput").ap()
    wkvk_l = nc.dram_tensor("wkvk_l", [128, CKV // 128, NHL * QK_NOPE], BF16,
                            kind="ExternalInput").ap()
    wkvv_l = nc.dram_tensor("wkvv_l", [128, CKV // 128, NHL * V_HEAD], BF16,
                            kind="ExternalInput").ap()
    wo_l = nc.dram_tensor("wo_l", [NHL * V_HEAD, H], BF16,
                          kind="ExternalInput").ap()
    out_p = nc.dram_tensor("out_p", [S, H], BF16, kind="ExternalOutput").ap()

    agin_kv = nc.dram_tensor("agin_kv", [CKV, RB], BF16,
                             kind="Internal").ap()
    agout_kv = nc.dram_tensor("agout_kv", [M * CKV, RB], BF16,
                              kind="Internal", addr_space="Shared").ap()
    agin_q = nc.dram_tensor("agin_q", [QLR + QK_ROPE, RB], BF16,
                            kind="Internal").ap()
    agout_q = nc.dram_tensor("agout_q", [M * (QLR + QK_ROPE), RB], BF16,
                             kind="Internal", addr_space="Shared").ap()
    agv_kv = agout_kv.rearrange("(r n) c -> n r c", r=M)
    agv_q = agout_q.rearrange("(r n) c -> n r c", r=M)

    with ExitStack() as ctx:
        tc = ctx.enter_context(tile.TileContext(nc))

        const = ctx.enter_context(tc.tile_pool(name="const", bufs=1))
        persist = ctx.enter_context(tc.tile_pool(name="persist", bufs=1))
        actx = ExitStack()
        sba = actx.enter_context(tc.tile_pool(name="sba", bufs=1))
        sbw = actx.enter_context(tc.tile_pool(name="sbw", bufs=2))
        ps = ctx.enter_context(tc.tile_pool(name="ps", bufs=2, space="PSUM"))

        # ---- phase-A constants (needed immediately) ----
        ident = const.tile([128, 128], BF16)
        make_identity(nc, ident)
        eps_t = const.tile([128, 1], F32)
        nc.vector.memset(eps_t, EPS)
        # natural-block cos/ssin [128, 2, 2*32], host-packed contiguous
        csb = const.tile([128, 2, 2 * QK_ROPE], F32)
        nc.scalar.dma_start(out=csb, in_=csb_h)

        # ---- persistent K^T and V' ----
        KT = [persist.tile([Q_HEAD, S], BF16, tag=f"KT{h}", name=f"KT{h}")
              for h in range(NHL)]
        Vp = persist.tile([128, NKT * VROW], BF16, tag="Vp")
        # ones columns of V' (once)
        nc.vector.memset(Vp, 1.0)

        # ================= PHASE A =================
        # hs^T first (gates everything)
        hsT = []
        for rt in range(2):
            hst = sba.tile([128, H], BF16, tag="hs", bufs=1, name=f"hs{rt}")
            nc.sync.dma_start(out=hst, in_=hs_b[128 * rt:128 * rt + 128, :])
            for hc in range(20):
                tp = ps.tile([128, 128], BF16, tag="work")
                nc.tensor.transpose(tp, hst[:, 128 * hc:128 * hc + 128], ident)
                t = sba.tile([128, 128], BF16, tag=f"hsT{rt}_{hc}",
                             name=f"hsT{rt}_{hc}")
                if hc % 2 == 0:
                    nc.vector.tensor_copy(t, tp)
                else:
                    nc.scalar.copy(t, tp)
                hsT.append(t)

        # ---- kv path first: ckv matmuls -> norm -> transpose -> AllGather
        # single-shot resident weight loads on separate queues (no chunk stalls)
        wkva_all = sba.tile([128, 20, CKV + QK_ROPE], BF16, tag="wkva_all",
                            bufs=1)
        nc.scalar.dma_start(out=wkva_all, in_=wkv_a)
        wqa_all = sba.tile([128, 20, QLR], BF16, tag="wqa_all", bufs=1)
        nc.gpsimd.dma_start(out=wqa_all, in_=wq_a)

        ckv_ps = [ps.tile([128, CKV + QK_ROPE], F32, tag=f"acc{4 + rt}", bufs=1,
                          name=f"ckv_ps{rt}")
                  for rt in range(2)]
        for hc in range(20):
            st, sp = hc == 0, hc == 19
            for rt in range(2):
                nc.tensor.matmul(ckv_ps[rt], hsT[rt * 20 + hc],
                                 wkva_all[:, hc, :], start=st, stop=sp)
        for rt in range(2):
            # --- ckv rms norm on first 256 cols ---
            sq3 = sbw.tile([128, CKV], F32, tag="sq", name=f"sq3_{rt}")
            ac = sbw.tile([128, 1], F32, tag="a0", name=f"ac_{rt}")
            nc.scalar.activation(sq3, ckv_ps[rt][:, 0:CKV], AF.Square, accum_out=ac)
            nc.scalar.activation(ac, ac, AF.Sqrt, bias=eps_t, scale=1.0 / CKV)
            crstd = sbw.tile([128, 1], F32, tag="a1", name=f"crstd_{rt}")
            nc.vector.reciprocal(crstd, ac)
            ckvn = sbw.tile([128, CKV], BF16, tag="ckvn", name=f"ckvn_{rt}")
            nc.vector.tensor_scalar_mul(ckvn, ckv_ps[rt][:, 0:CKV], crstd)
            for jc in range(2):
                tp = ps.tile([128, 128], BF16, tag="work", name=f"tpc_{rt}_{jc}")
                nc.tensor.transpose(tp, ckvn[:, 128 * jc:128 * jc + 128], ident)
                pc = sbw.tile([128, 128], BF16, tag="piece", name=f"pcc_{rt}_{jc}")
                nc.scalar.copy(pc, tp)
                nc.scalar.dma_start(out=agin_kv[128 * jc:128 * jc + 128,
                                              128 * rt:128 * rt + 128], in_=pc)

        nc.gpsimd.collective_compute(
            "AllGather", mybir.AluOpType.bypass,
            replica_groups=[list(range(M))],
            ins=[agin_kv], outs=[agout_kv],
        )

        # --- k_pe RoPE (natural) then transpose, into agin_q rows 768:800 ---
        for rt in range(2):
            t1 = sbw.tile([128, QK_ROPE], F32, tag="kp1", name=f"kp1_{rt}")
            nc.vector.tensor_mul(t1, ckv_ps[rt][:, CKV:CKV + QK_ROPE],
                                 csb[:, rt, 0:QK_ROPE])
            t2 = sbw.tile([128, QK_ROPE], F32, tag="kp2", name=f"kp2_{rt}")
            nc.vector.tensor_mul(t2[:, 0:16], ckv_ps[rt][:, CKV + 16:CKV + 32],
                                 csb[:, rt, QK_ROPE:QK_ROPE + 16])
            nc.vector.tensor_mul(t2[:, 16:32], ckv_ps[rt][:, CKV:CKV + 16],
                                 csb[:, rt, QK_ROPE + 16:QK_ROPE + 32])
            kpe = sbw.tile([128, QK_ROPE], BF16, tag="kp3", name=f"kp3_{rt}")
            nc.vector.tensor_add(kpe, t1, t2)
            tp = ps.tile([128, 128], BF16, tag="work", name=f"tpk_{rt}")
            nc.tensor.transpose(tp[0:QK_ROPE, :], kpe, ident)
            pc = sbw.tile([QK_ROPE, 128], BF16, tag="pieceb", name=f"pck_{rt}")
            nc.vector.tensor_copy(pc, tp[0:QK_ROPE, :])
            nc.scalar.dma_start(out=agin_q[QLR:QLR + QK_ROPE,
                                           128 * rt:128 * rt + 128], in_=pc)

        # ---- q path: qa matmuls -> norm -> transpose -> AllGather
        qa_ps = [[ps.tile([128, 384], F32, tag=f"acc{rt * 2 + jt}", bufs=1,
                          name=f"qa_ps{rt}{jt}")
                  for jt in range(2)] for rt in range(2)]
        for hc in range(20):
            st, sp = hc == 0, hc == 19
            for rt in range(2):
                for jt in range(2):
                    nc.tensor.matmul(
                        qa_ps[rt][jt], hsT[rt * 20 + hc],
                        wqa_all[:, hc, 384 * jt:384 * jt + 384],
                        start=st, stop=sp)

        for rt in range(2):
            # --- q_a rms norm (natural layout) ---
            sq = sbw.tile([128, 384], F32, tag="sq", name=f"sq_{rt}")
            a0 = sbw.tile([128, 1], F32, tag="a0", name=f"a0_{rt}")
            a1 = sbw.tile([128, 1], F32, tag="a1", name=f"a1_{rt}")
            nc.scalar.activation(sq, qa_ps[rt][0], AF.Square, accum_out=a0)
            sq2 = sbw.tile([128, 384], F32, tag="sq", name=f"sq2_{rt}")
            nc.scalar.activation(sq2, qa_ps[rt][1], AF.Square, accum_out=a1)
            ssum = sbw.tile([128, 1], F32, tag="a0", name=f"ssum_{rt}")
            nc.vector.tensor_add(ssum, a0, a1)
            nc.scalar.activation(ssum, ssum, AF.Sqrt, bias=eps_t, scale=1.0 / QLR)
            rstd = sbw.tile([128, 1], F32, tag="a1", name=f"rstd_{rt}")
            nc.vector.reciprocal(rstd, ssum)
            qan = sbw.tile([128, QLR], BF16, tag="qan", bufs=1, name=f"qan_{rt}")
            for jt in range(2):
                nc.vector.tensor_scalar_mul(qan[:, 384 * jt:384 * jt + 384],
                                            qa_ps[rt][jt], rstd)
            for jc in range(6):
                tp = ps.tile([128, 128], BF16, tag="work", name=f"tpq_{rt}_{jc}")
                nc.tensor.transpose(tp, qan[:, 128 * jc:128 * jc + 128], ident)
                pc = sbw.tile([128, 128], BF16, tag="pieceb",
                              name=f"pcq_{rt}_{jc}")
                if jc % 2 == 0:
                    nc.vector.tensor_copy(pc, tp)
                else:
                    nc.scalar.copy(pc, tp)
                nc.scalar.dma_start(out=agin_q[128 * jc:128 * jc + 128,
                                             128 * rt:128 * rt + 128], in_=pc)

        nc.gpsimd.collective_compute(
            "AllGather", mybir.AluOpType.bypass,
            replica_groups=[list(range(M))],
            ins=[agin_q], outs=[agout_q],
        )

        # ---- phase B/C weights: loaded while the AllGathers run ----
        tri_sb = const.tile([128, 128], BF16)
        nc.sync.dma_start(out=tri_sb, in_=tri)
        csT = const.tile([64, S], F32)
        nc.sync.dma_start(out=csT[0:32, :], in_=cosT)
        nc.sync.dma_start(out=csT[32:64, :], in_=ssinT)
        wkvk_sb = const.tile([128, 2, NHL * QK_NOPE], BF16)
        nc.sync.dma_start(out=wkvk_sb, in_=wkvk_l)
        wkvv_sb = const.tile([128, 2, NHL * V_HEAD], BF16)
        nc.sync.dma_start(out=wkvv_sb, in_=wkvv_l)
        wqb_sb = const.tile([128, 6, NHL * 128], BF16)
        nc.gpsimd.dma_start(out=wqb_sb, in_=wqb_l)
        wo01_sb = const.tile([128, 2, H], BF16)
        nc.gpsimd.dma_start(out=wo01_sb,
                            in_=wo_l[0:256].rearrange("(d p) c -> p d c", p=128))
        wo2_sb = const.tile([QK_NOPE, H], BF16)
        nc.gpsimd.dma_start(out=wo2_sb, in_=wo_l[256:320])

        actx.close()
        sbc = ctx.enter_context(tc.tile_pool(name="sbc", bufs=2))

        # ================= PHASE B: K^T and V' =================
        for kb in range(NQB):
            ckt = sbc.tile([128, 2, 2, RB], BF16, tag="latB", bufs=2,
                           name=f"ckt{kb}")
            for d in range(2):
                nc.sync.dma_start(
                    out=ckt[:, d],
                    in_=agv_kv[128 * d:128 * d + 128, 2 * kb:2 * kb + 2, :])
            cks = [ckt[:, 0], ckt[:, 1]]
            for h in range(NHL):
                kps = ps.tile([QK_NOPE, 512], F32, tag="work")
                for c in range(2):
                    nc.tensor.matmul(
                        kps,
                        wkvk_sb[:, c, QK_NOPE * h:QK_NOPE * h + QK_NOPE],
                        cks[c].rearrange("p r c -> p (r c)"),
                        start=(c == 0), stop=(c == 1))
                nc.scalar.copy(KT[h][0:QK_NOPE, 512 * kb:512 * kb + 512], kps)
                nc.gpsimd.dma_start(
                    out=KT[h][QK_NOPE:Q_HEAD, 512 * kb:512 * kb + 512]
                        .rearrange("p (r c) -> p r c", r=2),
                    in_=agv_q[QLR:QLR + QK_ROPE, 2 * kb:2 * kb + 2, :])
            for t4 in range(4):
                vps = ps.tile([128, NHL * V_HEAD], F32, tag="work")
                for c in range(2):
                    nc.tensor.matmul(
                        vps,
                        cks[c].rearrange("p r c -> p (r c)")
                              [:, 128 * t4:128 * t4 + 128],
                        wkvv_sb[:, c, :],
                        start=(c == 0), stop=(c == 1))
                kt = 4 * kb + t4
                vdst = bass.AP(tensor=Vp.tensor,
                               offset=Vp.offset + kt * VROW,
                               ap=[Vp.ap[0], [V_HEAD + 1, NHL], [1, V_HEAD]])
                nc.vector.tensor_copy(vdst, vps)

        # ================= PHASE C: per q-block =================
        QTs = {}
        LATs = {}

        def make_lat(qb):
            latt = sbc.tile([128, 6, 2, RB], BF16, tag="latC", bufs=2,
                            name=f"latt{qb}")
            for r in range(2):
                (nc.sync if r == 0 else nc.scalar).dma_start(
                    out=latt[:, :, r, :],
                    in_=agv_q[0:QLR].rearrange("(d p) r c -> p d r c", p=128)
                        [:, :, 2 * qb + r, :])
            LATs[qb] = [latt[:, c] for c in range(6)]
            QTs[qb] = []

        def make_qps(qb, h):
            # dedicated per-head accumulator tag: freed by att[h]'s epilogue
            # mul (the true dependency), not by the work-slot rotation whose
            # release rides unrelated ACT-counter waits.
            lats = LATs[qb]
            qps = ps.tile([128, 512], F32, tag=f"acc{h}", bufs=1,
                          name=f"qps{qb}_{h}")
            for c in range(6):
                nc.tensor.matmul(
                    qps, wqb_sb[:, c, 128 * h:128 * h + 128],
                    lats[c].rearrange("p r c -> p (r c)"),
                    start=(c == 0), stop=(c == 5))
            return qps

        def extract_qt(qb, h, qps):
            qs = slice(512 * qb, 512 * qb + 512)
            qt = sbc.tile([Q_HEAD, 512], BF16, tag="QT", bufs=10,
                          name=f"qt{qb}_{h}")
            nc.scalar.copy(qt[0:QK_NOPE, :], qps[0:QK_NOPE, :])
            t1 = sbc.tile([QK_ROPE, 512], F32, tag="rp1", bufs=2,
                          name=f"rp1_{qb}_{h}")
            nc.vector.tensor_mul(t1, qps[64:96, :], csT[0:32, qs])
            t2 = sbc.tile([QK_ROPE, 512], F32, tag="rp2", bufs=2,
                          name=f"rp2_{qb}_{h}")
            nc.vector.tensor_mul(t2, qps[96:128, :], csT[32:64, qs])
            nc.vector.tensor_add(qt[QK_NOPE:Q_HEAD, :], t1, t2)
            QTs[qb].append(qt)

        def make_qt(qb):
            make_lat(qb)
            for h in range(NHL):
                extract_qt(qb, h, make_qps(qb, h))

        _wk = [0]
        make_qt(0)
        for qb in range(NQB):
            QT = QTs[qb]
            att = [ps.tile([VROW // NHL, 512], F32, tag=f"acc{h}", bufs=1,
                           name=f"att{h}")
                   for h in range(NHL)]
            nkt = 4 * qb + 4
            for kt in range(nkt):
                o = max(0, 128 * kt - 512 * qb)
                for h in range(NHL):
                    wi = _wk[0]; _wk[0] += 1
                    sps = ps.tile([128, 512], F32,
                                  tag=("work" if wi % 3 < 2 else "acc5"),
                                  bufs=(2 if wi % 3 < 2 else 1),
                                  name=f"sps{qb}_{kt}_{h}")
                    nc.tensor.matmul(sps[:, o:512],
                                     KT[h][:, 128 * kt:128 * kt + 128],
                                     QT[h][:, o:512],
                                     start=True, stop=True)
                    pt = sbc.tile([128, 512], BF16, tag="PT", bufs=4,
                                  name=f"pt{qb}_{kt}_{h}")
                    nc.scalar.activation(pt[:, o:512], sps[:, o:512],
                                         AF.Exp, scale=SM_SCALE)
                    if 128 * kt >= 512 * qb:
                        nc.vector.tensor_mul(pt[:, o:o + 128],
                                             pt[:, o:o + 128], tri_sb)
                    nc.tensor.matmul(att[h][:, o:512],
                                     Vp[:, kt * VROW + 65 * h:kt * VROW + 65 * h + 65],
                                     pt[:, o:512],
                                     start=(kt == 0), stop=(kt == nkt - 1),
                                     skip_group_check=True)

            if qb + 1 < NQB:
                make_lat(qb + 1)

            # epilogue: softmax denominators -> normalized aT (bf16).
            # Denominator copies ride the (idle) Scalar engine; DVE keeps the
            # reciprocal + normalize muls; next q-block's QT matmuls fill the
            # PE, with the DVE-side qt extraction deferred two heads so aT[0]
            # completes early and the wo matmuls can start.
            aT = [sbc.tile([128, 512], BF16, tag=f"aT{p}", bufs=2, name=f"aT{p}")
                  for p in range(2)]
            aT4 = sbc.tile([QK_NOPE, 512], BF16, tag="aT4", bufs=2)
            qps_next = []
            for h in range(NHL):
                rcs = sbc.tile([1, 512], F32, tag="rcs", bufs=2,
                               name=f"rcs{qb}_{h}")
                nc.scalar.copy(rcs, att[h][64:65, :])
                rc = sbc.tile([1, 512], F32, tag="rc", bufs=2,
                              name=f"rc{qb}_{h}")
                nc.vector.reciprocal_approx_fast(rc, rcs)
                bcst = sbc.tile([QK_NOPE, 512], F32, tag="bc", bufs=2,
                                name=f"bc{qb}_{h}")
                nc.gpsimd.partition_broadcast(bcst, rc)
                if h < 4:
                    dst = aT[h // 2][64 * (h % 2):64 * (h % 2) + 64, :]
                else:
                    dst = aT4
                nc.vector.tensor_mul(dst, att[h][0:64, :], bcst)
                if qb + 1 < NQB:
                    qps_next.append(make_qps(qb + 1, h))
                    if h >= 2:
                        extract_qt(qb + 1, h - 2, qps_next[h - 2])
            if qb + 1 < NQB:
                for h in range(NHL - 2, NHL):
                    extract_qt(qb + 1, h, qps_next[h])
            for hc in range(5):
                hcs = slice(512 * hc, 512 * hc + 512)
                w0, w1 = wo01_sb[:, 0, hcs], wo01_sb[:, 1, hcs]
                w2 = wo2_sb[:, hcs]
                for half in range(2):
                    osb = sbc.tile([128, 2, 512], BF16, tag="osb", bufs=2,
                                   name=f"osb{qb}_{hc}_{half}")
                    for qq in range(2):
                        qt4 = 2 * half + qq
                        qsl = slice(128 * qt4, 128 * qt4 + 128)
                        wi = _wk[0]; _wk[0] += 1
                        ops = ps.tile([128, 512], F32,
                                      tag=("work" if wi % 3 < 2 else "acc5"),
                                      bufs=(2 if wi % 3 < 2 else 1),
                                      name=f"ops{qb}_{hc}_{qt4}")
                        nc.tensor.matmul(ops, aT[0][:, qsl], w0,
                                         start=True, stop=False)
                        nc.tensor.matmul(ops, aT[1][:, qsl], w1,
                                         start=False, stop=False)
                        nc.tensor.matmul(ops, aT4[:, qsl], w2,
                                         start=False, stop=True)
                        if qt4 % 2 == 0:
                            nc.vector.tensor_copy(osb[:, qq, :], ops)
                        else:
                            nc.scalar.copy(osb[:, qq, :], ops)
                    (nc.sync if hc % 2 == 0 else nc.gpsimd).dma_start(
                        out=out_p.rearrange("(d p) c -> p d c", p=128)
                                 [:, 4 * qb + 2 * half:4 * qb + 2 * half + 2, hcs],
                        in_=osb)

    nc.compile()
    return nc


def _prep(inputs):
    import ml_dtypes
    BF = ml_dtypes.bfloat16
    hs = np.ascontiguousarray(np.asarray(inputs["hidden_states"], np.float32)[0])
    cos = np.asarray(inputs["cos"], np.float32)
    sin = np.asarray(inputs["sin"], np.float32)
    wq_a = np.asarray(inputs["wq_a"], np.float32)
    q_ln = np.asarray(inputs["q_a_ln_w"], np.float32)
    wq_b = np.asarray(inputs["wq_b"], np.float32)
    wkv_a = np.asarray(inputs["wkv_a"], np.float32)
    kv_ln = np.asarray(inputs["kv_a_ln_w"], np.float32)
    wkv_b = np.asarray(inputs["wkv_b"], np.float32)
    wo = np.asarray(inputs["wo"], np.float32)

    if not np.all(q_ln == 1.0):
        wq_b = wq_b * q_ln[:, None]
    if not np.all(kv_ln == 1.0):
        wkv_b = wkv_b * kv_ln[:, None]

    ssin = np.concatenate([-sin[:, :16], sin[:, 16:]], axis=1)
    cosT = np.ascontiguousarray(cos.T)
    ssinT = np.ascontiguousarray(ssin.T)
    tri = np.triu(np.ones((128, 128), np.float32)).astype(BF)
    hs_bf = hs.astype(BF)

    def pm(w, t):
        # [t*128, c] row-major -> partition-major [128, t, c]
        c = w.shape[1]
        return np.ascontiguousarray(
            w.reshape(t, 128, c).transpose(1, 0, 2))

    wq_a_bf = pm(wq_a.astype(BF), 20)
    wkv_a_bf = pm(wkv_a.astype(BF), 20)

    in_maps = []
    for c in range(M):
        heads = range(NHL * c, NHL * c + NHL)
        qb_cols = []
        for h in heads:
            qb_cols.extend(range(96 * h, 96 * h + 96))
            # swapped pe columns: [16:32] then [0:16] of the pe block
            qb_cols.extend(range(96 * h + 80, 96 * h + 96))
            qb_cols.extend(range(96 * h + 64, 96 * h + 80))
        wqb_loc = pm(np.ascontiguousarray(wq_b[:, qb_cols]).astype(BF), 6)
        kcols, vcols = [], []
        for h in heads:
            kcols.extend(range(128 * h, 128 * h + 64))
            vcols.extend(range(128 * h + 64, 128 * h + 128))
        csb_loc = np.empty((128, 2, 2 * QK_ROPE), np.float32)
        cb = cos[RB * c:RB * c + RB].reshape(2, 128, QK_ROPE)
        sb = ssin[RB * c:RB * c + RB].reshape(2, 128, QK_ROPE)
        csb_loc[:, :, :QK_ROPE] = cb.transpose(1, 0, 2)
        csb_loc[:, :, QK_ROPE:] = sb.transpose(1, 0, 2)
        in_maps.append({
            "hs_b": np.ascontiguousarray(hs_bf[RB * c:RB * c + RB]),
            "csb_h": csb_loc,
            "cosT": cosT,
            "ssinT": ssinT,
            "tri": tri,
            "wq_a": wq_a_bf,
            "wkv_a": wkv_a_bf,
            "wqb_l": wqb_loc,
            "wkvk_l": pm(np.ascontiguousarray(wkv_b[:, kcols]).astype(BF), 2),
            "wkvv_l": pm(np.ascontiguousarray(wkv_b[:, vcols]).astype(BF), 2),
            "wo_l": np.ascontiguousarray(
                wo[NHL * V_HEAD * c:NHL * V_HEAD * (c + 1)]).astype(BF),
        })
    return in_maps


def kernel(**inputs):
    if "nc" not in _cache:
        _cache["nc"] = _build()
    nc = _cache["nc"]
    in_maps = _prep(inputs)
    res = run_bass_kernel_spmd(nc, in_maps, core_ids=list(range(M)))
    out = res.results[0]["out_p"].astype(np.float32)
    for c in range(1, M):
        out += res.results[c]["out_p"].astype(np.float32)
    return out.reshape(1, S, H)
